# revision 11
# baseline (speedup 1.0000x reference)
"""Trainium2 Bass kernel: dense multi-head dot-product attention.

Problem: x [4, 2048, 1024], W_Q/W_K/W_V [16, 1024, 64] ->
         out [4, 2048, 1024] (heads concatenated on the feature dim).

Sharding: 8 cores = 4 batches x 2 head-groups (8 heads each).
Per core, everything is computed in "transposed" layouts so that no
on-chip transpose of the big attention matrix is ever needed:
  - host passes x^T [1024, 2048] (n on partitions) per batch
  - projections (W stationary): Q^T/K^T/V^T [heads*64, 2048]
  - scores S^T[k, m] = sum_d K^T[d,k] Q^T[d,m]  (k on partitions)
  - P^T = exp(S^T/8)  (elementwise, ScalarE, PSUM->SBUF)
  - O^T[d, m] = sum_k Vaug[k, d] P^T[k, m] with Vaug = [V | ones],
    so row 64 of the accumulator is the softmax denominator.
  - normalize: recip(row64) (DVE) broadcast over partitions (GpSimd),
    multiply on GpSimd; output O^T [512, 2048] per core; host
    transposes when gathering.
Softmax skips the max-subtraction: |S/8| < ~16 here, exp is safe in
fp32 and softmax is shift-invariant, so the result is identical.

Matmul operands are float32r (fp32 bits, PE rounds internally,
~1e-4 rel err, 1 cyc/row at free-dim 512). PSUM stays fp32.

Pipeline structure (the point of this version):
  - All input DMAs are issued from the Sync engine (HWDGE): DRAM
    tensors are declared float32r so no SWDGE cast path is needed.
    W(pair0) + first x m-quarter go first so the PE starts ~10us in.
  - The attention inner loop is software-pipelined with a one-step
    delay on the AV matmul: slot kc emits scores(kc), exp(kc),
    AV(kc-1).  exp(kc-1) ran while the PE did scores(kc), so the AV
    never waits on the Scalar engine.
  - Projections + V-transposes for pair p+1 are interleaved into the
    attention phase of pair p (~2 PE instructions per kc slot), so the
    Scalar engine's exp stream keeps running during what used to be
    PE-only projection phases.
  - PSUM: st 2x[128,1024] (4 banks) + ot 1x[65,1024] (2 banks) +
    proj/transpose 2x[128,512] (2 banks) = 8 banks.
"""

from contextlib import ExitStack

import numpy as np

import concourse.bass as bass  # noqa: F401  (bass types via bacc)
import concourse.tile as tile
from concourse import bacc, mybir
from concourse import bass_utils
from concourse.masks import make_identity

F32 = mybir.dt.float32
F32R = mybir.dt.float32r

B, M, N, H, D = 4, 2048, 1024, 16, 64
HPC = 8          # heads per core
NCORES = 8
NCH = 8          # d_model / 128 chunks
KC = 16          # key chunks of 128
SCALE = 0.125    # 1/sqrt(64)
MH = 1024        # m-half width


def build_nc():
    nc = bacc.Bacc(
        "TRN2", target_bir_lowering=False, debug=False, enable_asserts=False
    )
    xt_d = nc.dram_tensor("xt", [N, M], F32R, kind="ExternalInput")
    wq_d = nc.dram_tensor("wq", [4, N, 128], F32R, kind="ExternalInput")
    wk_d = nc.dram_tensor("wk", [4, N, 128], F32R, kind="ExternalInput")
    wv_d = nc.dram_tensor("wv", [4, N, 128], F32R, kind="ExternalInput")
    o_d = nc.dram_tensor("ot", [HPC * D, M], F32, kind="ExternalOutput")
    w_drams = {"q": wq_d, "k": wk_d, "v": wv_d}

    with tile.TileContext(nc) as tc, ExitStack() as ctx:
        const_pool = ctx.enter_context(tc.tile_pool(name="constp", bufs=1))
        xt_pool = ctx.enter_context(tc.tile_pool(name="xtp", bufs=NCH))
        w_pool = ctx.enter_context(tc.tile_pool(name="wp", bufs=2))
        qkv_pool = ctx.enter_context(tc.tile_pool(name="qkvp", bufs=2))
        vaug_pool = ctx.enter_context(tc.tile_pool(name="vaugp", bufs=2))
        pt_pool = ctx.enter_context(tc.tile_pool(name="ptp", bufs=2))
        out_pool = ctx.enter_context(tc.tile_pool(name="outp", bufs=3))
        small_pool = ctx.enter_context(tc.tile_pool(name="smallp", bufs=3))
        st_pool = ctx.enter_context(tc.tile_pool(name="stp", bufs=2, space="PSUM"))
        ot_pool = ctx.enter_context(tc.tile_pool(name="otp", bufs=1, space="PSUM"))
        pj_pool = ctx.enter_context(tc.tile_pool(name="pjp", bufs=2, space="PSUM"))

        ident = const_pool.tile([128, 128], F32, name="ident")
        make_identity(nc, ident[:])
        ones16 = const_pool.tile([128, 16, 1], F32, name="ones16")
        nc.gpsimd.memset(ones16[:], 1.0)
        zero64 = const_pool.tile([64, 512], F32, name="zero64")
        nc.gpsimd.memset(zero64[:], 0.0)

        # Persistent per-head K tiles, zero-padded to the full 128
        # contraction rows: kt0 holds head0's K in rows 0:64 (rows 64:128
        # stay zero), kt1 holds head1's K in rows 64:128.  The score
        # matmul then always runs with a [128, 128] stationary tile —
        # same PE array configuration as every other matmul.  Mixing
        # 64-row and 128-row stationaries forces a PE reconfiguration
        # that costs ~200ns per transition (~2 per kc slot).
        # The zero rows are written once here; projection copies only
        # ever touch the head's own rows.  Manually ping-ponged (a/b).
        kts = []
        for hp in range(2):
            pair_tiles = []
            for ab in range(2):
                ktile = const_pool.tile(
                    [128, M], F32R, name=f"kt{hp}{ab}"
                )
                zsl = slice(64, 128) if hp == 0 else slice(0, 64)
                for blk in range(4):
                    nc.vector.tensor_copy(
                        ktile[zsl, blk * 512:(blk + 1) * 512], zero64[:]
                    )
                pair_tiles.append(ktile)
            kts.append(pair_tiles)

        # ---- resident x^T tiles; first m-quarter issued before pair-0 W
        # so the first projection matmuls can start after ~3.5MB of DMA.
        xts = []
        for c in range(NCH):
            xtile = xt_pool.tile([128, M], F32R, name=f"xt{c}", tag="xtile")
            xts.append(xtile)

        def dma_w(p):
            wts = {}
            for nm, wd in w_drams.items():
                wt = w_pool.tile(
                    [128, NCH, 128], F32R, name=f"wt_{nm}", tag=f"wt_{nm}"
                )
                nc.sync.dma_start(
                    wt[:], wd.ap()[p].rearrange("(c p) d -> p c d", p=128)
                )
                wts[nm] = wt
            return wts

        wts0 = dma_w(0)
        for c in range(NCH):
            nc.sync.dma_start(
                xts[c][:, 0:512], xt_d.ap()[c * 128:(c + 1) * 128, 0:512]
            )

        def make_pair(p, wts):
            """Allocate pair p's QKV/vaug tiles; return (state, generator).
            Each next() on the generator emits ~one PE instruction of the
            projection/transpose work for this pair."""
            st = {}
            st["q"] = qkv_pool.tile([128, M], F32R, name="qt", tag="qt")
            st["k0"] = kts[0][p % 2]
            st["k1"] = kts[1][p % 2]
            st["v"] = qkv_pool.tile([128, M], F32, name="vt", tag="vt")
            st["vaug"] = vaug_pool.tile(
                [128, KC, 130], F32R, name="vaug", tag="vaug"
            )

            def gen():
                # projections, m-block major so pair 0 tracks the x DMA
                # quarters arriving in order
                for blk in range(4):
                    bsl = slice(blk * 512, (blk + 1) * 512)
                    for nm in ("q", "k", "v"):
                        ps = pj_pool.tile([128, 512], F32, name="pjps", tag="pj")
                        for c in range(NCH):
                            nc.tensor.matmul(
                                ps[:],
                                lhsT=wts[nm][:, c, :],
                                rhs=xts[c][:, blk * 512:(blk + 1) * 512],
                                start=(c == 0),
                                stop=(c == NCH - 1),
                                skip_group_check=True,
                            )
                            yield
                        if nm == "k":
                            nc.vector.tensor_copy(
                                st["k0"][0:64, bsl], ps[0:64, :]
                            )
                            nc.vector.tensor_copy(
                                st["k1"][64:128, bsl], ps[64:128, :]
                            )
                        else:
                            nc.vector.tensor_copy(st[nm][:, bsl], ps[:])
                # Vaug[k, kc, hp*65 + d]; col hp*65+64 = 1.0 (ones col)
                vaug = st["vaug"]
                for hp in range(2):
                    nc.vector.tensor_copy(
                        vaug[:, :, hp * 65 + 64:hp * 65 + 65], ones16[:]
                    )
                for kc in range(KC):
                    trp = pj_pool.tile([128, 128], F32, name="trp", tag="pj")
                    nc.tensor.transpose(
                        trp[:], st["v"][:, kc * 128:(kc + 1) * 128], ident[:]
                    )
                    yield
                    nc.vector.tensor_copy(
                        vaug[:, kc, :].rearrange("p (h x) -> p h x", h=2)[
                            :, :, 0:64
                        ],
                        trp.rearrange("p (h d) -> p h d", h=2),
                    )

            return st, gen()

        def emit_drain(h, mbase, ot):
            # NOTE: keep this the hardware-proven shape — the reciprocal
            # custom-DVE op reads from a partition-0 [1, MH] tile (a
            # partition-64-offset input broke on HW while passing CoreSim).
            sumsb = small_pool.tile([1, MH], F32, name="sumsb", tag="sm")
            nc.vector.tensor_copy(sumsb[:], ot[64:65, :])
            ostage = out_pool.tile([64, MH], F32, name="ostage", tag="o64")
            nc.vector.tensor_copy(ostage[:], ot[0:64, :])
            recipb = small_pool.tile([1, MH], F32, name="recipb", tag="sm")
            scratch = small_pool.tile([1, MH], F32, name="scr", tag="sm")
            nc.vector.reciprocal_approx_accurate(recipb[:], sumsb[:], scratch[:])
            rbc = out_pool.tile([64, MH], F32, name="rbc", tag="o64")
            nc.gpsimd.partition_broadcast(rbc[:], recipb[:])
            stage = out_pool.tile([64, MH], F32, name="stage", tag="o64")
            nc.vector.tensor_mul(stage[:], ostage[:], rbc[:])
            nc.sync.dma_start(
                o_d.ap()[h * 64:(h + 1) * 64, mbase:mbase + MH], stage[:]
            )

        def run_attention(p, state, next_gen):
            qt, vaug = state["q"], state["vaug"]
            pending = None
            box = {"ot": None}
            for hp in range(2):
                h = 2 * p + hp
                kt = state[f"k{hp}"]
                for mh in range(2):
                    mbase = mh * MH
                    for kc in range(KC):
                        stt = st_pool.tile([128, MH], F32, name="st", tag="st")
                        for mc in range(2):
                            nc.tensor.matmul(
                                stt[:, mc * 512:(mc + 1) * 512],
                                lhsT=kt[:, kc * 128:(kc + 1) * 128],
                                rhs=qt[
                                    :,
                                    mbase + mc * 512: mbase + (mc + 1) * 512,
                                ],
                                start=True,
                                stop=True,
                            )
                        pt = pt_pool.tile([128, MH], F32R, name="pt", tag="pt")
                        nc.scalar.activation(
                            pt[:], stt[:],
                            mybir.ActivationFunctionType.Exp, scale=SCALE,
                        )

                        if pending is not None:
                            pending()
                        if next_gen is not None:
                            next(next_gen, None)
                            next(next_gen, None)

                        def make_av(hp=hp, h=h, mbase=mbase, kc=kc, pt=pt):
                            def av():
                                if kc == 0:
                                    box["ot"] = ot_pool.tile(
                                        [65, MH], F32, name="ot", tag="ot"
                                    )
                                ot = box["ot"]
                                for mc in range(2):
                                    nc.tensor.matmul(
                                        ot[:, mc * 512:(mc + 1) * 512],
                                        lhsT=vaug[:, kc, hp * 65:hp * 65 + 65],
                                        rhs=pt[:, mc * 512:(mc + 1) * 512],
                                        start=(kc == 0),
                                        stop=(kc == KC - 1),
                                        skip_group_check=True,
                                    )
                                if kc == KC - 1:
                                    emit_drain(h, mbase, ot)
                            return av

                        pending = make_av()
            # flush the last AV + its drain
            pending()
            return next_gen

        # rest of x; queued behind W0 + q0 so those still arrive first
        for q in range(1, 4):
            for c in range(NCH):
                nc.sync.dma_start(
                    xts[c][:, q * 512:(q + 1) * 512],
                    xt_d.ap()[c * 128:(c + 1) * 128, q * 512:(q + 1) * 512],
                )

        # ---- pair 0 projections run standalone (pipeline fill)
        state, gen = make_pair(0, wts0)
        for _ in gen:
            pass

        for p in range(4):
            if p < 3:
                wts_n = dma_w(p + 1)
                state_n, gen_n = make_pair(p + 1, wts_n)
            else:
                state_n, gen_n = None, None
            leftover = run_attention(p, state, gen_n)
            if leftover is not None:
                for _ in leftover:
                    pass
            state = state_n
    nc.compile()
    return nc


_NC_CACHE = None


def _get_nc():
    global _NC_CACHE
    if _NC_CACHE is None:
        _NC_CACHE = build_nc()
    return _NC_CACHE


def make_in_maps(x, W_Q, W_K, W_V):
    x = np.asarray(x, dtype=np.float32)
    W_Q = np.asarray(W_Q, dtype=np.float32)
    W_K = np.asarray(W_K, dtype=np.float32)
    W_V = np.asarray(W_V, dtype=np.float32)

    def prep_w(W, g):
        blk = W[8 * g:8 * g + 8]  # [8, 1024, 64]
        # pair-major [4, 1024, 128]: col = (head%2)*64 + d
        return np.ascontiguousarray(
            blk.reshape(4, 2, N, D).transpose(0, 2, 1, 3).reshape(4, N, 2 * D)
        )

    in_maps = []
    for c in range(NCORES):
        b, g = divmod(c, 2)
        in_maps.append(
            {
                "xt": np.ascontiguousarray(x[b].T),
                "wq": prep_w(W_Q, g),
                "wk": prep_w(W_K, g),
                "wv": prep_w(W_V, g),
            }
        )
    return in_maps


def gather_out(results):
    out = np.empty((B, M, N), dtype=np.float32)
    for c in range(NCORES):
        b, g = divmod(c, 2)
        out[b, :, 512 * g:512 * (g + 1)] = results[c]["ot"].T
    return out


def run(x, W_Q, W_K, W_V, **spmd_kwargs):
    nc = _get_nc()
    in_maps = make_in_maps(x, W_Q, W_K, W_V)
    res = bass_utils.run_bass_kernel_spmd(
        nc, in_maps, core_ids=list(range(NCORES)), **spmd_kwargs
    )
    return gather_out(res.results), res


def kernel(x, W_Q, W_K, W_V):
    out, _ = run(x, W_Q, W_K, W_V)
    return out


# revision 18
# speedup vs baseline: 1.2572x; 1.2572x over previous
"""Trainium2 Bass kernel: dense multi-head dot-product attention.

Problem: x [4, 2048, 1024], W_Q/W_K/W_V [16, 1024, 64] ->
         out [4, 2048, 1024] (heads concatenated on the feature dim).

Sharding: 8 cores = 4 batches x 2 head-groups (8 heads each).
Per core, everything is computed in "transposed" layouts so that no
on-chip transpose of the big attention matrix is ever needed:
  - host passes x^T [1024, 2048] (n on partitions) per batch
  - projections (W stationary): Q^T/K^T/V^T [heads*64, 2048]
  - scores S^T[k, m] = sum_d K^T[d,k] Q^T[d,m]  (k on partitions)
  - P^T = exp(S^T/8)  (elementwise, ScalarE, PSUM->SBUF)
  - O^T[d, m] = sum_k Vaug[k, d] P^T[k, m] with Vaug = [V | ones],
    so row 64 of the accumulator is the softmax denominator.
  - normalize: recip(row64) (DVE) broadcast over partitions (GpSimd),
    multiply on GpSimd; output O^T [512, 2048] per core; host
    transposes when gathering.
Softmax skips the max-subtraction: |S/8| < ~16 here, exp is safe in
fp32 and softmax is shift-invariant, so the result is identical.

Matmul operands are float32r (fp32 bits, PE rounds internally,
~1e-4 rel err, 1 cyc/row at free-dim 512). PSUM stays fp32.

Pipeline structure (the point of this version):
  - All input DMAs are issued from the Sync engine (HWDGE): DRAM
    tensors are declared float32r so no SWDGE cast path is needed.
    W(pair0) + first x m-quarter go first so the PE starts ~10us in.
  - The attention inner loop is software-pipelined with a one-step
    delay on the AV matmul: slot kc emits scores(kc), exp(kc),
    AV(kc-1).  exp(kc-1) ran while the PE did scores(kc), so the AV
    never waits on the Scalar engine.
  - Projections + V-transposes for pair p+1 are interleaved into the
    attention phase of pair p (~2 PE instructions per kc slot), so the
    Scalar engine's exp stream keeps running during what used to be
    PE-only projection phases.
  - PSUM: st 2x[128,1024] (4 banks) + ot 1x[65,1024] (2 banks) +
    proj/transpose 2x[128,512] (2 banks) = 8 banks.
"""

from contextlib import ExitStack

import numpy as np

import concourse.bass as bass  # noqa: F401  (bass types via bacc)
import concourse.tile as tile
from concourse import bacc, mybir
from concourse import bass_utils
from concourse.masks import make_identity

F32 = mybir.dt.float32
F32R = mybir.dt.float32r

B, M, N, H, D = 4, 2048, 1024, 16, 64
HPC = 8          # heads per core
NCORES = 8
NCH = 8          # d_model / 128 chunks
KC = 16          # key chunks of 128
SCALE = 0.125    # 1/sqrt(64)
MH = 1024        # m-half width


def build_nc():
    nc = bacc.Bacc(
        "TRN2", target_bir_lowering=False, debug=False, enable_asserts=False
    )
    xt_d = nc.dram_tensor("xt", [N, M], F32R, kind="ExternalInput")
    wq_d = nc.dram_tensor("wq", [4, N, 128], F32R, kind="ExternalInput")
    wk_d = nc.dram_tensor("wk", [4, N, 128], F32R, kind="ExternalInput")
    wv_d = nc.dram_tensor("wv", [4, N, 128], F32R, kind="ExternalInput")
    o_d = nc.dram_tensor("ot", [HPC * D, M], F32, kind="ExternalOutput")
    w_drams = {"q": wq_d, "k": wk_d, "v": wv_d}

    with tile.TileContext(nc) as tc, ExitStack() as ctx:
        const_pool = ctx.enter_context(tc.tile_pool(name="constp", bufs=1))
        xt_pool = ctx.enter_context(tc.tile_pool(name="xtp", bufs=NCH))
        w_pool = ctx.enter_context(tc.tile_pool(name="wp", bufs=2))
        qkv_pool = ctx.enter_context(tc.tile_pool(name="qkvp", bufs=2))
        vaug_pool = ctx.enter_context(tc.tile_pool(name="vaugp", bufs=2))
        pt_pool = ctx.enter_context(tc.tile_pool(name="ptp", bufs=3))
        out_pool = ctx.enter_context(tc.tile_pool(name="outp", bufs=3))
        small_pool = ctx.enter_context(tc.tile_pool(name="smallp", bufs=3))
        st_pool = ctx.enter_context(tc.tile_pool(name="stp", bufs=2, space="PSUM"))
        ot_pool = ctx.enter_context(tc.tile_pool(name="otp", bufs=1, space="PSUM"))
        pj_pool = ctx.enter_context(tc.tile_pool(name="pjp", bufs=2, space="PSUM"))

        ident = const_pool.tile([128, 128], F32, name="ident")
        make_identity(nc, ident[:])
        ones16 = const_pool.tile([128, 16, 1], F32, name="ones16")
        nc.gpsimd.memset(ones16[:], 1.0)
        zero64 = const_pool.tile([64, 128], F32, name="zero64")
        nc.gpsimd.memset(zero64[:], 0.0)

        # Persistent per-head K tiles, zero-padded to the full 128
        # contraction rows: kt0 holds head0's K in rows 0:64 (rows 64:128
        # stay zero), kt1 holds head1's K in rows 64:128.  The score
        # matmul then always runs with a [128, 128] stationary tile —
        # same PE array configuration as every other matmul.  Mixing
        # 64-row and 128-row stationaries forces a PE reconfiguration
        # that costs ~200ns per transition (~2 per kc slot).
        # The zero rows are written once here; projection copies only
        # ever touch the head's own rows.  Manually ping-ponged (a/b).
        kts = []
        for hp in range(2):
            pair_tiles = []
            for ab in range(2):
                ktile = const_pool.tile(
                    [128, M], F32R, name=f"kt{hp}{ab}"
                )
                zsl = slice(64, 128) if hp == 0 else slice(0, 64)
                for blk in range(16):
                    nc.vector.tensor_copy(
                        ktile[zsl, blk * 128:(blk + 1) * 128], zero64[:]
                    )
                pair_tiles.append(ktile)
            kts.append(pair_tiles)

        # ---- resident x^T tiles; first m-quarter issued before pair-0 W
        # so the first projection matmuls can start after ~3.5MB of DMA.
        xts = []
        for c in range(NCH):
            xtile = xt_pool.tile([128, M], F32R, name=f"xt{c}", tag="xtile")
            xts.append(xtile)

        def dma_w(p, names=("q", "k", "v")):
            wts = {}
            for nm in names:
                wd = w_drams[nm]
                wt = w_pool.tile(
                    [128, NCH, 128], F32R, name=f"wt_{nm}", tag=f"wt_{nm}"
                )
                nc.sync.dma_start(
                    wt[:], wd.ap()[p].rearrange("(c p) d -> p c d", p=128)
                )
                wts[nm] = wt
            return wts

        # wq first, then the first x m-quarter, then wk/wv: the first
        # projection matmul needs only wq + x quarter 0 (~2.5MB).
        wts0 = dma_w(0, names=("q",))
        for c in range(NCH):
            nc.sync.dma_start(
                xts[c][:, 0:512], xt_d.ap()[c * 128:(c + 1) * 128, 0:512]
            )
        wts0.update(dma_w(0, names=("k", "v")))

        def make_pair(p, wts):
            """Allocate pair p's QKV/vaug tiles; return (state, generator).
            Each next() on the generator emits ~one PE instruction of the
            projection/transpose work for this pair."""
            st = {}
            st["q"] = qkv_pool.tile([128, M], F32R, name="qt", tag="qt")
            st["k0"] = kts[0][p % 2]
            st["k1"] = kts[1][p % 2]
            st["v"] = qkv_pool.tile([128, M], F32, name="vt", tag="vt")
            st["vaug"] = vaug_pool.tile(
                [128, KC, 130], F32R, name="vaug", tag="vaug"
            )

            def gen():
                # projections, m-block major so pair 0 tracks the x DMA
                # quarters arriving in order
                for blk in range(4):
                    bsl = slice(blk * 512, (blk + 1) * 512)
                    for nm in ("q", "k", "v"):
                        ps = pj_pool.tile([128, 512], F32, name="pjps", tag="pj")
                        for c in range(NCH):
                            nc.tensor.matmul(
                                ps[:],
                                lhsT=wts[nm][:, c, :],
                                rhs=xts[c][:, blk * 512:(blk + 1) * 512],
                                start=(c == 0),
                                stop=(c == NCH - 1),
                                skip_group_check=True,
                            )
                            yield
                        if nm == "k":
                            nc.vector.tensor_copy(
                                st["k0"][0:64, bsl], ps[0:64, :]
                            )
                            nc.vector.tensor_copy(
                                st["k1"][64:128, bsl], ps[64:128, :]
                            )
                        else:
                            nc.vector.tensor_copy(st[nm][:, bsl], ps[:])
                # Vaug[k, kc, hp*65 + d]; col hp*65+64 = 1.0 (ones col)
                vaug = st["vaug"]
                for hp in range(2):
                    nc.vector.tensor_copy(
                        vaug[:, :, hp * 65 + 64:hp * 65 + 65], ones16[:]
                    )
                for kc in range(KC):
                    trp = pj_pool.tile([128, 128], F32, name="trp", tag="pj")
                    nc.tensor.transpose(
                        trp[:], st["v"][:, kc * 128:(kc + 1) * 128], ident[:]
                    )
                    yield
                    nc.vector.tensor_copy(
                        vaug[:, kc, :].rearrange("p (h x) -> p h x", h=2)[
                            :, :, 0:64
                        ],
                        trp.rearrange("p (h d) -> p h d", h=2),
                    )

            return st, gen()

        def emit_drain(h, mbase, ot):
            # NOTE: keep this the hardware-proven shape — the reciprocal
            # custom-DVE op reads from a partition-0 [1, MH] tile (a
            # partition-64-offset input broke on HW while passing CoreSim).
            sumsb = small_pool.tile([1, MH], F32, name="sumsb", tag="sm")
            nc.vector.tensor_copy(sumsb[:], ot[64:65, :])
            ostage = out_pool.tile([64, MH], F32, name="ostage", tag="o64")
            nc.vector.tensor_copy(ostage[:], ot[0:64, :])
            recipb = small_pool.tile([1, MH], F32, name="recipb", tag="sm")
            scratch = small_pool.tile([1, MH], F32, name="scr", tag="sm")
            nc.vector.reciprocal_approx_accurate(recipb[:], sumsb[:], scratch[:])
            rbc = out_pool.tile([64, MH], F32, name="rbc", tag="o64")
            nc.gpsimd.partition_broadcast(rbc[:], recipb[:])
            stage = out_pool.tile([64, MH], F32, name="stage", tag="o64")
            nc.vector.tensor_mul(stage[:], ostage[:], rbc[:])
            nc.sync.dma_start(
                o_d.ap()[h * 64:(h + 1) * 64, mbase:mbase + MH], stage[:]
            )

        def run_attention(p, state, next_gen):
            qt, vaug = state["q"], state["vaug"]
            pending = []   # AV closures, emitted with a 2-slot delay
            box = {"ot": None}
            for hp in range(2):
                h = 2 * p + hp
                kt = state[f"k{hp}"]
                for mh in range(2):
                    mbase = mh * MH
                    for kc in range(KC):
                        stt = st_pool.tile([128, MH], F32, name="st", tag="st")
                        for mc in range(2):
                            nc.tensor.matmul(
                                stt[:, mc * 512:(mc + 1) * 512],
                                lhsT=kt[:, kc * 128:(kc + 1) * 128],
                                rhs=qt[
                                    :,
                                    mbase + mc * 512: mbase + (mc + 1) * 512,
                                ],
                                start=True,
                                stop=True,
                            )
                        pt = pt_pool.tile([128, MH], F32R, name="pt", tag="pt")
                        nc.scalar.activation(
                            pt[:], stt[:],
                            mybir.ActivationFunctionType.Exp, scale=SCALE,
                        )

                        if len(pending) >= 2:
                            pending.pop(0)()
                        if next_gen is not None:
                            next(next_gen, None)
                            next(next_gen, None)

                        def make_av(hp=hp, h=h, mbase=mbase, kc=kc, pt=pt):
                            def av():
                                if kc == 0:
                                    box["ot"] = ot_pool.tile(
                                        [65, MH], F32, name="ot", tag="ot"
                                    )
                                ot = box["ot"]
                                for mc in range(2):
                                    nc.tensor.matmul(
                                        ot[:, mc * 512:(mc + 1) * 512],
                                        lhsT=vaug[:, kc, hp * 65:hp * 65 + 65],
                                        rhs=pt[:, mc * 512:(mc + 1) * 512],
                                        start=(kc == 0),
                                        stop=(kc == KC - 1),
                                        skip_group_check=True,
                                    )
                                if kc == KC - 1:
                                    emit_drain(h, mbase, ot)
                            return av

                        pending.append(make_av())
            # flush the trailing AVs + their drains
            for av in pending:
                av()
            return next_gen

        # rest of x; queued behind W0 + q0 so those still arrive first
        for q in range(1, 4):
            for c in range(NCH):
                nc.sync.dma_start(
                    xts[c][:, q * 512:(q + 1) * 512],
                    xt_d.ap()[c * 128:(c + 1) * 128, q * 512:(q + 1) * 512],
                )

        # ---- pair 0 projections run standalone (pipeline fill)
        state, gen = make_pair(0, wts0)
        for _ in gen:
            pass

        for p in range(4):
            if p < 3:
                wts_n = dma_w(p + 1)
                state_n, gen_n = make_pair(p + 1, wts_n)
            else:
                state_n, gen_n = None, None
            leftover = run_attention(p, state, gen_n)
            if leftover is not None:
                for _ in leftover:
                    pass
            state = state_n
    nc.compile()
    return nc


_NC_CACHE = None


def _get_nc():
    global _NC_CACHE
    if _NC_CACHE is None:
        _NC_CACHE = build_nc()
    return _NC_CACHE


def make_in_maps(x, W_Q, W_K, W_V):
    x = np.asarray(x, dtype=np.float32)
    W_Q = np.asarray(W_Q, dtype=np.float32)
    W_K = np.asarray(W_K, dtype=np.float32)
    W_V = np.asarray(W_V, dtype=np.float32)

    def prep_w(W, g):
        blk = W[8 * g:8 * g + 8]  # [8, 1024, 64]
        # pair-major [4, 1024, 128]: col = (head%2)*64 + d
        return np.ascontiguousarray(
            blk.reshape(4, 2, N, D).transpose(0, 2, 1, 3).reshape(4, N, 2 * D)
        )

    in_maps = []
    for c in range(NCORES):
        b, g = divmod(c, 2)
        in_maps.append(
            {
                "xt": np.ascontiguousarray(x[b].T),
                "wq": prep_w(W_Q, g),
                "wk": prep_w(W_K, g),
                "wv": prep_w(W_V, g),
            }
        )
    return in_maps


def gather_out(results):
    out = np.empty((B, M, N), dtype=np.float32)
    for c in range(NCORES):
        b, g = divmod(c, 2)
        out[b, :, 512 * g:512 * (g + 1)] = results[c]["ot"].T
    return out


def run(x, W_Q, W_K, W_V, **spmd_kwargs):
    nc = _get_nc()
    in_maps = make_in_maps(x, W_Q, W_K, W_V)
    res = bass_utils.run_bass_kernel_spmd(
        nc, in_maps, core_ids=list(range(NCORES)), **spmd_kwargs
    )
    return gather_out(res.results), res


def kernel(x, W_Q, W_K, W_V):
    out, _ = run(x, W_Q, W_K, W_V)
    return out


# revision 26
# speedup vs baseline: 1.2726x; 1.0123x over previous
"""Trainium2 Bass kernel: dense multi-head dot-product attention.

Problem: x [4, 2048, 1024], W_Q/W_K/W_V [16, 1024, 64] ->
         out [4, 2048, 1024] (heads concatenated on the feature dim).

Sharding: 8 cores = 4 batches x 2 head-groups (8 heads each).
Per core, everything is computed in "transposed" layouts so that no
on-chip transpose of the big attention matrix is ever needed:
  - host passes x^T [1024, 2048] (n on partitions) per batch
  - projections (W stationary): Q^T/K^T/V^T [heads*64, 2048]
  - scores S^T[k, m] = sum_d K^T[d,k] Q^T[d,m]  (k on partitions)
  - P^T = exp(S^T/8)  (elementwise, ScalarE, PSUM->SBUF)
  - O^T[d, m] = sum_k Vaug[k, d] P^T[k, m] with Vaug = [V | ones],
    so row 64 of the accumulator is the softmax denominator.
  - normalize: recip(row64) (DVE) broadcast over partitions (GpSimd),
    multiply on GpSimd; output O^T [512, 2048] per core; host
    transposes when gathering.
Softmax skips the max-subtraction: |S/8| < ~16 here, exp is safe in
fp32 and softmax is shift-invariant, so the result is identical.

Matmul operands are float32r (fp32 bits, PE rounds internally,
~1e-4 rel err, 1 cyc/row at free-dim 512). PSUM stays fp32.

Pipeline structure (the point of this version):
  - All input DMAs are issued from the Sync engine (HWDGE): DRAM
    tensors are declared float32r so no SWDGE cast path is needed.
    W(pair0) + first x m-quarter go first so the PE starts ~10us in.
  - The attention inner loop is software-pipelined with a one-step
    delay on the AV matmul: slot kc emits scores(kc), exp(kc),
    AV(kc-1).  exp(kc-1) ran while the PE did scores(kc), so the AV
    never waits on the Scalar engine.
  - Projections + V-transposes for pair p+1 are interleaved into the
    attention phase of pair p (~2 PE instructions per kc slot), so the
    Scalar engine's exp stream keeps running during what used to be
    PE-only projection phases.
  - PSUM: st 2x[128,1024] (4 banks) + ot 1x[65,1024] (2 banks) +
    proj/transpose 2x[128,512] (2 banks) = 8 banks.
"""

from contextlib import ExitStack

import ml_dtypes
import numpy as np

import concourse.bass as bass  # noqa: F401  (bass types via bacc)
import concourse.tile as tile
from concourse import bacc, mybir
from concourse import bass_utils
from concourse.masks import make_identity

F32 = mybir.dt.float32
F32R = mybir.dt.float32r
BF16 = mybir.dt.bfloat16

B, M, N, H, D = 4, 2048, 1024, 16, 64
HPC = 8          # heads per core
NCORES = 8
NCH = 8          # d_model / 128 chunks
KC = 16          # key chunks of 128
SCALE = 0.125    # 1/sqrt(64)
MH = 1024        # m-half width


def build_nc():
    nc = bacc.Bacc(
        "TRN2", target_bir_lowering=False, debug=False, enable_asserts=False
    )
    # x and W arrive as bf16 (host-side cast): halves input DMA traffic.
    # bf16 matmuls run at the same 1 cyc/col as f32r; the ~0.4%
    # quantization keeps overall rel err well under the 2e-2 gate.
    xt_d = nc.dram_tensor("xt", [N, M], BF16, kind="ExternalInput")
    wq_d = nc.dram_tensor("wq", [4, N, 128], BF16, kind="ExternalInput")
    wk_d = nc.dram_tensor("wk", [4, N, 128], BF16, kind="ExternalInput")
    wv_d = nc.dram_tensor("wv", [4, N, 128], BF16, kind="ExternalInput")
    o_d = nc.dram_tensor("ot", [HPC * D, M], F32, kind="ExternalOutput")
    w_drams = {"q": wq_d, "k": wk_d, "v": wv_d}

    with tile.TileContext(nc) as tc, ExitStack() as ctx:
        const_pool = ctx.enter_context(tc.tile_pool(name="constp", bufs=1))
        xt_pool = ctx.enter_context(tc.tile_pool(name="xtp", bufs=NCH))
        w_pool = ctx.enter_context(tc.tile_pool(name="wp", bufs=2))
        qkv_pool = ctx.enter_context(tc.tile_pool(name="qkvp", bufs=2))
        vaug_pool = ctx.enter_context(tc.tile_pool(name="vaugp", bufs=2))
        pt_pool = ctx.enter_context(tc.tile_pool(name="ptp", bufs=3))
        out_pool = ctx.enter_context(tc.tile_pool(name="outp", bufs=3))
        small_pool = ctx.enter_context(tc.tile_pool(name="smallp", bufs=3))
        st_pool = ctx.enter_context(tc.tile_pool(name="stp", bufs=2, space="PSUM"))
        ot_pool = ctx.enter_context(tc.tile_pool(name="otp", bufs=1, space="PSUM"))
        pj_pool = ctx.enter_context(tc.tile_pool(name="pjp", bufs=2, space="PSUM"))

        ident = const_pool.tile([128, 128], F32, name="ident")
        make_identity(nc, ident[:])
        ones16 = const_pool.tile([128, 16, 1], F32, name="ones16")
        nc.gpsimd.memset(ones16[:], 1.0)
        zero64 = const_pool.tile([64, 128], F32, name="zero64")
        nc.gpsimd.memset(zero64[:], 0.0)

        # Persistent per-head K tiles, zero-padded to the full 128
        # contraction rows: kt0 holds head0's K in rows 0:64 (rows 64:128
        # stay zero), kt1 holds head1's K in rows 64:128.  The score
        # matmul then always runs with a [128, 128] stationary tile —
        # same PE array configuration as every other matmul.  Mixing
        # 64-row and 128-row stationaries forces a PE reconfiguration
        # that costs ~200ns per transition (~2 per kc slot).
        # The zero rows are written once here; projection copies only
        # ever touch the head's own rows.  Manually ping-ponged (a/b).
        kts = []
        for hp in range(2):
            pair_tiles = []
            for ab in range(2):
                ktile = const_pool.tile(
                    [128, M], F32R, name=f"kt{hp}{ab}"
                )
                zsl = slice(64, 128) if hp == 0 else slice(0, 64)
                for blk in range(16):
                    nc.vector.tensor_copy(
                        ktile[zsl, blk * 128:(blk + 1) * 128], zero64[:]
                    )
                pair_tiles.append(ktile)
            kts.append(pair_tiles)

        # ---- resident x^T tiles; first m-quarter issued before pair-0 W
        # so the first projection matmuls can start after ~3.5MB of DMA.
        xts = []
        for c in range(NCH):
            xtile = xt_pool.tile([128, M], BF16, name=f"xt{c}", tag="xtile")
            xts.append(xtile)

        def dma_w(p, names=("q", "k", "v")):
            wts = {}
            for nm in names:
                wd = w_drams[nm]
                wt = w_pool.tile(
                    [128, NCH, 128], BF16, name=f"wt_{nm}", tag=f"wt_{nm}"
                )
                nc.sync.dma_start(
                    wt[:], wd.ap()[p].rearrange("(c p) d -> p c d", p=128)
                )
                wts[nm] = wt
            return wts

        # wq first, then the first x m-quarter, then wk/wv: the first
        # projection matmul needs only wq + x quarter 0 (~2.5MB).
        wts0 = dma_w(0, names=("q",))
        for c in range(NCH):
            nc.sync.dma_start(
                xts[c][:, 0:512], xt_d.ap()[c * 128:(c + 1) * 128, 0:512]
            )
        wts0.update(dma_w(0, names=("k", "v")))

        def make_pair(p, wts):
            """Allocate pair p's QKV/vaug tiles; return (state, generator).
            Each next() on the generator emits ~one PE instruction of the
            projection/transpose work for this pair."""
            st = {}
            st["q"] = qkv_pool.tile([128, M], F32R, name="qt", tag="qt")
            st["k0"] = kts[0][p % 2]
            st["k1"] = kts[1][p % 2]
            st["v"] = qkv_pool.tile([128, M], F32, name="vt", tag="vt")
            st["vaug"] = vaug_pool.tile(
                [128, KC, 130], F32R, name="vaug", tag="vaug"
            )

            def gen():
                # projections, m-block major so pair 0 tracks the x DMA
                # quarters arriving in order
                for blk in range(4):
                    bsl = slice(blk * 512, (blk + 1) * 512)
                    for nm in ("q", "k", "v"):
                        ps = pj_pool.tile([128, 512], F32, name="pjps", tag="pj")
                        for c in range(NCH):
                            nc.tensor.matmul(
                                ps[:],
                                lhsT=wts[nm][:, c, :],
                                rhs=xts[c][:, blk * 512:(blk + 1) * 512],
                                start=(c == 0),
                                stop=(c == NCH - 1),
                                skip_group_check=True,
                            )
                            yield
                        if nm == "k":
                            nc.vector.tensor_copy(
                                st["k0"][0:64, bsl], ps[0:64, :]
                            )
                            nc.vector.tensor_copy(
                                st["k1"][64:128, bsl], ps[64:128, :]
                            )
                        else:
                            nc.vector.tensor_copy(st[nm][:, bsl], ps[:])
                # Vaug[k, kc, hp*65 + d]; col hp*65+64 = 1.0 (ones col)
                vaug = st["vaug"]
                for hp in range(2):
                    nc.vector.tensor_copy(
                        vaug[:, :, hp * 65 + 64:hp * 65 + 65], ones16[:]
                    )
                for kc in range(KC):
                    trp = pj_pool.tile([128, 128], F32, name="trp", tag="pj")
                    nc.tensor.transpose(
                        trp[:], st["v"][:, kc * 128:(kc + 1) * 128], ident[:]
                    )
                    yield
                    nc.vector.tensor_copy(
                        vaug[:, kc, :].rearrange("p (h x) -> p h x", h=2)[
                            :, :, 0:64
                        ],
                        trp.rearrange("p (h d) -> p h d", h=2),
                    )

            return st, gen()

        def emit_drain_split(h, mbase, ot):
            # Final drain: chunk the normalize chain so its serial latency
            # (copy->recip->broadcast->mul->dma) pipelines at the kernel tail.
            for q in range(2):
                qsl = slice(q * 512, (q + 1) * 512)
                sumsb = small_pool.tile([1, 512], F32, name="sumsbq", tag="smq")
                nc.vector.tensor_copy(sumsb[:], ot[64:65, qsl])
                ostage = out_pool.tile([64, 512], F32, name="ostageq", tag="o64q")
                nc.vector.tensor_copy(ostage[:], ot[0:64, qsl])
                recipb = small_pool.tile([1, 512], F32, name="recipbq", tag="smq")
                scratch = small_pool.tile([1, 512], F32, name="scrq", tag="smq")
                nc.vector.reciprocal_approx_accurate(
                    recipb[:], sumsb[:], scratch[:]
                )
                rbc = out_pool.tile([64, 512], F32, name="rbcq", tag="o64q")
                nc.gpsimd.partition_broadcast(rbc[:], recipb[:])
                stage = out_pool.tile([64, 512], F32, name="stageq", tag="o64q")
                nc.vector.tensor_mul(stage[:], ostage[:], rbc[:])
                nc.sync.dma_start(
                    o_d.ap()[
                        h * 64:(h + 1) * 64,
                        mbase + q * 512:mbase + (q + 1) * 512,
                    ],
                    stage[:],
                )

        def emit_drain(h, mbase, ot):
            # NOTE: keep this the hardware-proven shape — the reciprocal
            # custom-DVE op reads from a partition-0 [1, MH] tile (a
            # partition-64-offset input broke on HW while passing CoreSim).
            sumsb = small_pool.tile([1, MH], F32, name="sumsb", tag="sm")
            nc.vector.tensor_copy(sumsb[:], ot[64:65, :])
            ostage = out_pool.tile([64, MH], F32, name="ostage", tag="o64")
            nc.vector.tensor_copy(ostage[:], ot[0:64, :])
            recipb = small_pool.tile([1, MH], F32, name="recipb", tag="sm")
            scratch = small_pool.tile([1, MH], F32, name="scr", tag="sm")
            nc.vector.reciprocal_approx_accurate(recipb[:], sumsb[:], scratch[:])
            rbc = out_pool.tile([64, MH], F32, name="rbc", tag="o64")
            nc.gpsimd.partition_broadcast(rbc[:], recipb[:])
            stage = out_pool.tile([64, MH], F32, name="stage", tag="o64")
            nc.vector.tensor_mul(stage[:], ostage[:], rbc[:])
            nc.sync.dma_start(
                o_d.ap()[h * 64:(h + 1) * 64, mbase:mbase + MH], stage[:]
            )

        def run_attention(p, state, next_gen):
            qt, vaug = state["q"], state["vaug"]
            pending = []   # AV closures, emitted with a 2-slot delay
            box = {"ot": None}
            for hp in range(2):
                h = 2 * p + hp
                kt = state[f"k{hp}"]
                for mh in range(2):
                    mbase = mh * MH
                    for kc in range(KC):
                        stt = st_pool.tile([128, MH], F32, name="st", tag="st")
                        for mc in range(2):
                            nc.tensor.matmul(
                                stt[:, mc * 512:(mc + 1) * 512],
                                lhsT=kt[:, kc * 128:(kc + 1) * 128],
                                rhs=qt[
                                    :,
                                    mbase + mc * 512: mbase + (mc + 1) * 512,
                                ],
                                start=True,
                                stop=True,
                            )
                        pt = pt_pool.tile([128, MH], F32R, name="pt", tag="pt")
                        nc.scalar.activation(
                            pt[:], stt[:],
                            mybir.ActivationFunctionType.Exp, scale=SCALE,
                        )

                        if len(pending) >= 2:
                            pending.pop(0)()
                        if next_gen is not None:
                            next(next_gen, None)
                            next(next_gen, None)

                        def make_av(hp=hp, h=h, mbase=mbase, kc=kc, pt=pt):
                            def av():
                                if kc == 0:
                                    box["ot"] = ot_pool.tile(
                                        [65, MH], F32, name="ot", tag="ot"
                                    )
                                ot = box["ot"]
                                for mc in range(2):
                                    nc.tensor.matmul(
                                        ot[:, mc * 512:(mc + 1) * 512],
                                        lhsT=vaug[:, kc, hp * 65:hp * 65 + 65],
                                        rhs=pt[:, mc * 512:(mc + 1) * 512],
                                        start=(kc == 0),
                                        stop=(kc == KC - 1),
                                        skip_group_check=True,
                                    )
                                if kc == KC - 1:
                                    if p == 3 and hp == 1 and mh == 1:
                                        emit_drain_split(h, mbase, ot)
                                    else:
                                        emit_drain(h, mbase, ot)
                            return av

                        pending.append(make_av())
            # flush the trailing AVs + their drains
            for av in pending:
                av()
            return next_gen

        # rest of x; queued behind W0 + q0 so those still arrive first
        for q in range(1, 4):
            for c in range(NCH):
                nc.sync.dma_start(
                    xts[c][:, q * 512:(q + 1) * 512],
                    xt_d.ap()[c * 128:(c + 1) * 128, q * 512:(q + 1) * 512],
                )

        # ---- pair 0 projections run standalone (pipeline fill)
        state, gen = make_pair(0, wts0)
        for _ in gen:
            pass

        for p in range(4):
            if p < 3:
                wts_n = dma_w(p + 1)
                state_n, gen_n = make_pair(p + 1, wts_n)
            else:
                state_n, gen_n = None, None
            leftover = run_attention(p, state, gen_n)
            if leftover is not None:
                for _ in leftover:
                    pass
            state = state_n
    nc.compile()
    return nc


_NC_CACHE = None


def _get_nc():
    global _NC_CACHE
    if _NC_CACHE is None:
        _NC_CACHE = build_nc()
    return _NC_CACHE


def make_in_maps(x, W_Q, W_K, W_V):
    bf16 = ml_dtypes.bfloat16
    x = np.asarray(x, dtype=np.float32)
    W_Q = np.asarray(W_Q, dtype=np.float32)
    W_K = np.asarray(W_K, dtype=np.float32)
    W_V = np.asarray(W_V, dtype=np.float32)

    def prep_w(W, g):
        blk = W[8 * g:8 * g + 8]  # [8, 1024, 64]
        # pair-major [4, 1024, 128]: col = (head%2)*64 + d
        return np.ascontiguousarray(
            blk.reshape(4, 2, N, D).transpose(0, 2, 1, 3).reshape(4, N, 2 * D)
        ).astype(bf16)

    in_maps = []
    for c in range(NCORES):
        b, g = divmod(c, 2)
        in_maps.append(
            {
                "xt": np.ascontiguousarray(x[b].T).astype(bf16),
                "wq": prep_w(W_Q, g),
                "wk": prep_w(W_K, g),
                "wv": prep_w(W_V, g),
            }
        )
    return in_maps


def gather_out(results):
    out = np.empty((B, M, N), dtype=np.float32)
    for c in range(NCORES):
        b, g = divmod(c, 2)
        out[b, :, 512 * g:512 * (g + 1)] = results[c]["ot"].T
    return out


def run(x, W_Q, W_K, W_V, **spmd_kwargs):
    nc = _get_nc()
    in_maps = make_in_maps(x, W_Q, W_K, W_V)
    res = bass_utils.run_bass_kernel_spmd(
        nc, in_maps, core_ids=list(range(NCORES)), **spmd_kwargs
    )
    return gather_out(res.results), res


def kernel(x, W_Q, W_K, W_V):
    out, _ = run(x, W_Q, W_K, W_V)
    return out
